# revision 20
# baseline (speedup 1.0000x reference)
"""Trainium2 Bass kernel for nn_LDRLoss_80187039416585.

loss = mean(|LDR(pred_sq) - LDR(target_sq)|) where
LDR(x) = log(ema(x, c_short)) - log(roll(ema(x, c_long), -65047))

v3 strategy (8 cores, data-parallel over batch, 2 rows/core):
  - Each row of T=2^20 is processed as nh=2 "virtual rows" (halves) of
    128x4096: partition p of half h owns time [h*T/2 + p*4096, ... + 4096).
    Small halves shrink every SBUF tile, letting the log tiles triple-buffer
    so combine work can legally trail into later halves' emission (deep
    cross-half pipelining), and halve the pipeline fill/drain cost.
  - x is loaded once per half as bf16 via GPSIMD SWDGE cast-DMA.
  - Scale-free EMA y' = q*y' + x (c cancels in log differences). One DVE
    scan pass per stream per half; the per-partition initial states come
    from a PE carry matmul I = Mc @ f (+ v * W for halves h>0, where the
    scalar W chains the EMA state across the half boundary), read by the
    scan directly from PSUM.
  - "Finals" f (zero-init EMA endpoints per partition) are piecewise-
    constant-weight sums on ACT: chunked Copy(scale=chunk-mean q-power)
    + accum_out (1 whole-tile call for the long streams, 4 chunks for the
    short), tiny DVE fold. Finals+carries are emitted one vrow AHEAD of
    the scans (software pipelining) so PE carries never queue behind the
    previous vrow's combine matmuls and ACT drains finals during scans.
  - ACT applies Ln with a magnitude-centering scale, downcasting into bf16
    log tiles (the scale cancels in the combine differences).
  - Combine on PE with bf16 signed permutation matmuls into PSUM:
    +Sp - St - Lp_shift + Lt_shift. The 65047-sample roll crosses half
    boundaries, so each long-stream matmul splits into a low-partition part
    (same half) and a high-partition part (next half / row wrap); ACT fuses
    Abs + accumulate (partials) from PSUM.
  - Host sums partials (fp64) -> mean.  No cross-core traffic.
"""
import sys

sys.path.insert(0, "/opt/trn_rl_repo")

import numpy as np

import concourse.bacc as bacc
import concourse.tile as tile
from concourse import mybir

SR = 44100.0
QS = float(np.exp(-2200.0 / (50.0 * SR)))  # 1 - c_short
QL = float(np.exp(-2200.0 / (3000.0 * SR)))  # 1 - c_long
B, T = 16, 1 << 20
NCORES = 8
SHIFT = 65047

f32 = mybir.dt.float32
bf16 = mybir.dt.bfloat16
AL = mybir.AluOpType
AF = mybir.ActivationFunctionType
NPBF16 = mybir.dt.np(bf16)


class Cfg:
    """Geometry; small instances override for simulator tests."""

    def __init__(self, T=T, rpc=B // NCORES, shift=SHIFT, nh=2, fch=1024,
                 pech=512, dve_fin_chunks=2):
        self.P = 128
        self.T = T
        self.rpc = rpc
        self.nh = nh                 # halves (virtual rows) per input row
        self.F = T // (self.P * nh)  # segment length per partition
        self.shift = shift
        self.kp = shift // self.F
        self.rem = shift % self.F
        self.cut = self.F - self.rem
        self.pech = pech             # combine PSUM slot width
        # ACT finals: chunk width for the piecewise-constant weights
        self.fchw = min(512, self.F)
        self.nfch = self.F // self.fchw
        assert self.cut < pech, "boundary split must live in the first chunk"
        assert self.F % self.fchw == 0 and self.F % pech == 0
        assert self.kp < self.P
        assert nh <= 2, "W chain (wv.T @ f) is exact only for nh <= 2"
        # combine segments: (col_start, col_end, extra_shift) where the
        # partition shift is kp + extra_shift
        segs = [(0, self.cut, 0)]
        x0 = self.cut
        first = min((self.cut // pech + 1) * pech, self.F)
        segs.append((x0, first, 1))
        x0 = first
        while x0 < self.F:
            segs.append((x0, x0 + pech, 1))
            x0 += pech
        self.segs = segs
        self.nslots = len(segs)
        self.nvr = rpc * nh          # virtual rows


def _carry_matrix(q, F, P=128):
    """lhsT[k, p] = (q^F)^(p-1-k) for k < p; I = lhsT.T @ f."""
    lA = F * np.log(np.float64(q))
    M = np.zeros((P, P), dtype=np.float64)
    for p in range(1, P):
        ks = np.arange(p)
        M[:p, p] = np.exp(lA * (p - 1 - ks))
    return M.astype(np.float32)


def _wvec(q, F, P=128):
    """w[k, 0] = (q^F)^(127-k): W = w.T @ f = chain state of a full half."""
    lA = F * np.log(np.float64(q))
    return np.exp(lA * (P - 1 - np.arange(P, dtype=np.float64))) \
        .astype(np.float32).reshape(P, 1)


def _vvec(q, F, P=128):
    """v[0, p] = (q^F)^p: I_extra[p] = v.T-row * W (K=1 matmul lhsT)."""
    lA = F * np.log(np.float64(q))
    return np.exp(lA * np.arange(P, dtype=np.float64)) \
        .astype(np.float32).reshape(1, P)


def _perm_split(ks, sign, part, P=128):
    """Split circular permutation: out[m] += sign * src[(m+ks)] with the
    source taken from the same half (low, m+ks < P) or the next half /
    row-wrap (high, m+ks >= P)."""
    M = np.zeros((P, P), dtype=np.float32)
    for m in range(P):
        j = m + ks
        if part == "low" and j < P:
            M[j, m] = sign
        elif part == "high" and j >= P:
            M[j - P, m] = sign
    return M.astype(NPBF16)


def _perm(sign, P=128):
    return (sign * np.eye(P, dtype=np.float32)).astype(NPBF16)


def _fin_chunk_means(q, F, fchw):
    """Exact chunk means of q^(F-1-j) over contiguous fchw-wide chunks."""
    w = np.exp(np.arange(F - 1, -1, -1, dtype=np.float64)
               * np.log(np.float64(q)))
    return w.reshape(F // fchw, fchw).mean(axis=1).astype(np.float32)


def build(cfg: Cfg):
    P, F = cfg.P, cfg.F
    PECH = cfg.pech
    SCALE_S = float(2.0 * (1.0 - QS))
    SCALE_L = float(2.0 * (1.0 - QL))

    nc = bacc.Bacc("TRN2", target_bir_lowering=False, debug=False,
                   num_devices=NCORES)
    xp_d = nc.dram_tensor("xp", [cfg.rpc, cfg.T], f32, kind="ExternalInput")
    xt_d = nc.dram_tensor("xt", [cfg.rpc, cfg.T], f32, kind="ExternalInput")
    part_d = nc.dram_tensor("partials", [P, cfg.nvr * cfg.nslots], f32,
                            kind="ExternalOutput")

    mcs_d = nc.inline_tensor(_carry_matrix(QS, F), name="mcs")
    mcl_d = nc.inline_tensor(_carry_matrix(QL, F), name="mcl")
    wvs_d = nc.inline_tensor(_wvec(QS, F), name="wvs")
    wvl_d = nc.inline_tensor(_wvec(QL, F), name="wvl")
    vvs_d = nc.inline_tensor(_vvec(QS, F), name="vvs")
    vvl_d = nc.inline_tensor(_vvec(QL, F), name="vvl")
    cIp_d = nc.inline_tensor(_perm(1.0), name="cIp")
    cIn_d = nc.inline_tensor(_perm(-1.0), name="cIn")
    perm_d = {}
    for ks_extra in (0, 1):
        for sign, snm in ((-1.0, "n"), (1.0, "p")):
            for part in ("low", "high"):
                key = (ks_extra, snm, part)
                perm_d[key] = nc.inline_tensor(
                    _perm_split(cfg.kp + ks_extra, sign, part),
                    name=f"pm{ks_extra}{snm}{part[0]}")
    WMEAN_S = _fin_chunk_means(QS, F, max(F // 4, 1))
    WMEAN_L = _fin_chunk_means(QL, F, F)

    with tile.TileContext(nc) as tc:
        with (
            tc.tile_pool(name="consts", bufs=1) as consts,
            tc.tile_pool(name="xpool", bufs=2) as xpool,
            tc.tile_pool(name="logs_s", bufs=2) as logs_s,
            tc.tile_pool(name="logs_l", bufs=3) as logs_l,
            tc.tile_pool(name="scratch", bufs=4) as scratch,
            tc.tile_pool(name="fins", bufs=2) as fins,
            tc.tile_pool(name="psc", bufs=3, space="PSUM") as psc,
            tc.tile_pool(name="psi", bufs=2, space="PSUM") as psi,
            tc.tile_pool(name="psw", bufs=1, space="PSUM") as psw,
        ):
            def cload(d, shape, dt=f32):
                t = consts.tile(shape, dt, tag=d.name, name=d.name + "_t")
                nc.sync.dma_start(t[:], d[:])
                return t

            # x cast-DMA loads (bf16) on one SWDGE queue; v0 is emitted
            # before the (2 MB) weight constants so the first stream's
            # input wins the serialized DMA engines
            xts = {}

            def load_x(v):
                if v in xts or v >= cfg.nvr:
                    return
                r, h = divmod(v, cfg.nh)
                xpt = xpool.tile([P, F], bf16, tag="xp", name=f"xp_{v}")
                xtt = xpool.tile([P, F], bf16, tag="xt", name=f"xt_{v}")
                for d_t, t in ((xp_d, xpt), (xt_d, xtt)):
                    src = d_t[r].rearrange("(h p f) -> h p f", h=cfg.nh, p=P)
                    nc.gpsimd.dma_start(t[:], src[h])
                xts[v] = (xpt, xtt)

            load_x(0)

            # load order = first-use order: the long streams' carry matrix
            # gates the very first carry/scan; the combine's permutation
            # matrices aren't needed for ~50us
            mcl_t = cload(mcl_d, [P, P])
            wvl_t = cload(wvl_d, [P, 1])
            vvl_t = cload(vvl_d, [1, P])
            mcs_t = cload(mcs_d, [P, P])
            wvs_t = cload(wvs_d, [P, 1])
            vvs_t = cload(vvs_d, [1, P])
            cIp_t = cload(cIp_d, [P, P], bf16)
            cIn_t = cload(cIn_d, [P, P], bf16)
            perm_t = {k: cload(d, [P, P], bf16) for k, d in perm_d.items()}

            qs_t = consts.tile([P, 1], f32, tag="qs")
            nc.vector.memset(qs_t[:], QS)
            ql_t = consts.tile([P, 1], f32, tag="ql")
            nc.vector.memset(ql_t[:], QL)

            partials = consts.tile([P, cfg.nvr * cfg.nslots], f32, tag="part")
            # shared write-only dump for the finals Copy outputs (never
            # read; all writers are in-order ACT calls, so one buffer
            # suffices and frees SBUF for a deeper ssc rotation)
            fsc = consts.tile([P, F], bf16, tag="fscdump")

            def streams(v):
                xpt, xtt = xts[v]
                return (
                    ("Lp", xpt, ql_t, WMEAN_L, mcl_t, wvl_t, vvl_t,
                     SCALE_L),
                    ("Lt", xtt, ql_t, WMEAN_L, mcl_t, wvl_t, vvl_t,
                     SCALE_L),
                    ("Sp", xpt, qs_t, WMEAN_S, mcs_t, wvs_t, vvs_t,
                     SCALE_S),
                    ("St", xtt, qs_t, WMEAN_S, mcs_t, wvs_t, vvs_t,
                     SCALE_S),
                )

            def emit_passA(v, prev_W):
                """Finals (ACT chunked Copy+accum) + PE carry matmuls for
                all 4 streams of vrow v. Emitted one vrow AHEAD of pass B
                so carries never queue behind the previous vrow's combine
                matmuls on PE, and ACT drains finals while DVE scans."""
                r, h = divmod(v, cfg.nh)
                Ws = {}
                f4 = fins.tile([P, 4], f32, tag="f4", name=f"f4_{v}")
                wps4 = psw.tile([1, 4], f32, tag="wps", name=f"wps_{v}") \
                    if h < cfg.nh - 1 else None
                ips4 = psi.tile([P, 4], f32, tag="ips", name=f"ips_{v}")
                for si, (key, x_t, q_t, wmean, mc_t, wv_t, vv_t,
                         sc) in enumerate(streams(v)):
                    # ---- finals f = sum_j q^(F-1-j) x[:, j] ----
                    # Piecewise-constant block weights: ACT Copy with
                    # scale=chunk-mean(q^(F-1-j)) + accum_out gives the
                    # chunk's weighted sum on the Scalar engine (DVE and
                    # GPSIMD stay free for scans / DMA). The deterministic
                    # part of the block-weight approximation cancels by
                    # construction (exact chunk means); the stochastic
                    # residual is ~0.2% of f, invisible in the metric.
                    nch = len(wmean)
                    chw = F // nch
                    if nch == 1:
                        # long streams: one whole-tile call, accum straight
                        # into f4 (weight varies only q_l^F ~ 6.6% across
                        # the tile; the chunk-mean makes the deterministic
                        # part exact)
                        nc.scalar.activation(
                            fsc[:], x_t[:], AF.Copy, bias=0.0,
                            scale=float(wmean[0]), accum_out=f4[:, si:si + 1])
                    else:
                        facc = fins.tile([P, nch], f32, tag=f"facc{si}",
                                         name=f"facc_{key}_{v}")
                        for c in range(nch):
                            a, b = c * chw, (c + 1) * chw
                            nc.scalar.activation(
                                fsc[:, a:b], x_t[:, a:b], AF.Copy,
                                bias=0.0, scale=float(wmean[c]),
                                accum_out=facc[:, c:c + 1])
                        gsc8 = fins.tile([P, nch], bf16, tag=f"gsc8{si}",
                                         name=f"gsc8_{key}_{v}")
                        # fold on ACT (right behind the finals in its
                        # queue): keeps the DVE queue pure scans, so a
                        # vrow's scans never sit behind the NEXT vrow's
                        # folds waiting on its x DMA / finals
                        nc.scalar.activation(
                            gsc8[:], facc[:], AF.Copy, bias=0.0, scale=1.0,
                            accum_out=f4[:, si:si + 1])
                    # ---- carry: I = Mc @ f (+ v * W_prev for h > 0) ----
                    ips = ips4[:, si:si + 1]
                    chain = h > 0
                    nc.tensor.matmul(ips, mc_t[:], f4[:, si:si + 1],
                                     start=True, stop=not chain)
                    if chain:
                        nc.tensor.matmul(ips, vv_t[:],
                                         prev_W[key][0:1, :],
                                         start=False, stop=True)
                    if h < cfg.nh - 1:
                        # W = wv.T @ f: chain state at the end of this half
                        # (exact for nh=2); PSUM->SBUF copy on ACT.
                        nc.tensor.matmul(wps4[0:1, si:si + 1], wv_t[:],
                                         f4[:, si:si + 1])
                        wsb = fins.tile([1, 1], f32, tag=f"wsb{key}",
                                        name=f"wsb_{key}_{v}")
                        nc.scalar.copy(wsb[:], wps4[0:1, si:si + 1])
                        Ws[key] = wsb

                return Ws, ips4

            def emit_passB(v, ips4, logt, ln_cb):
                for si, (key, x_t, q_t, wmean, mc_t, wv_t, vv_t,
                         sc) in enumerate(streams(v)):
                    ips = ips4[:, si:si + 1]
                    # ---- scan + Ln (last stream split for early combine) --
                    ssc = scratch.tile([P, F], f32, tag="ssc",
                                       name=f"ssc_{key}_{v}")
                    # the very last stream of the kernel gets fine chunking
                    # so the final combine's slots un-gate progressively
                    nchunks = 1 if si < 3 else (8 if v == cfg.nvr - 1 else 2)
                    w = F // nchunks
                    for c in range(nchunks):
                        init = ips if c == 0 else ssc[:, c * w - 1:c * w]
                        nc.vector.tensor_tensor_scan(
                            ssc[:, c * w:(c + 1) * w],
                            q_t[:].broadcast_to([P, w]),
                            x_t[:, c * w:(c + 1) * w],
                            init, AL.mult, AL.add)
                        nc.scalar.activation(
                            logt[key][:, c * w:(c + 1) * w],
                            ssc[:, c * w:(c + 1) * w], AF.Ln, scale=sc)
                        ln_cb(si, (c + 1) * w if si == 3 else 0)

            def make_combiner(v, logt, logt_other):
                """Combine for virtual row v. logt_other = log tiles of the
                half the roll shift spills into ((r,h+1), or (r,0) wrap)."""
                base = v * cfg.nslots
                cursor = [0]

                def emit_up_to(max_slots, bound=None):
                    si = cursor[0]
                    n = 0
                    while si < cfg.nslots and n < max_slots and (
                            bound is None or cfg.segs[si][1] <= bound):
                        # pack 2 PECH-wide slots into one 2-bank PSUM tile
                        take = 1
                        ps = psc.tile([P, PECH], f32, tag="psc",
                                      name=f"ps_{v}_{si}")
                        for k in range(take):
                            a, b, ke = cfg.segs[si + k]
                            w = b - a
                            off = k * PECH
                            if ke == 0:
                                s0, s1 = a + cfg.rem, b + cfg.rem
                            else:
                                s0, s1 = a - cfg.cut, b - cfg.cut
                            mm = nc.tensor.matmul
                            mm(ps[:, off:off + w], cIp_t[:],
                               logt["Sp"][:, a:b], start=True, stop=False)
                            mm(ps[:, off:off + w], cIn_t[:],
                               logt["St"][:, a:b], start=False, stop=False)
                            mm(ps[:, off:off + w], perm_t[(ke, "n", "low")][:],
                               logt["Lp"][:, s0:s1], start=False, stop=False)
                            mm(ps[:, off:off + w], perm_t[(ke, "p", "low")][:],
                               logt["Lt"][:, s0:s1], start=False, stop=False)
                            mm(ps[:, off:off + w],
                               perm_t[(ke, "n", "high")][:],
                               logt_other["Lp"][:, s0:s1],
                               start=False, stop=False)
                            mm(ps[:, off:off + w],
                               perm_t[(ke, "p", "high")][:],
                               logt_other["Lt"][:, s0:s1],
                               start=False, stop=True)
                        for k in range(take):
                            a, b, _ = cfg.segs[si + k]
                            w = b - a
                            off = k * PECH
                            slot = base + si + k
                            nc.scalar.activation(
                                ps[:, off:off + w], ps[:, off:off + w],
                                AF.Abs, accum_out=partials[:, slot:slot + 1])
                        si += take
                        n += take
                    cursor[0] = si

                emit_up_to.done = lambda: cursor[0] >= cfg.nslots
                return emit_up_to

            all_logt = []
            pending = []           # combiners with all inputs emitted
            deferred = []          # (v, logt) waiting for their other-half
            load_x(1)
            stateA = {0: emit_passA(0, None)}
            for v in range(cfg.nvr):
                r, h = divmod(v, cfg.nh)
                load_x(v + 1)
                if v + 1 < cfg.nvr:
                    stateA[v + 1] = emit_passA(v + 1, stateA[v][0])
                logt = {
                    key: (logs_s if key[0] == "S" else logs_l).tile(
                        [P, F], bf16, tag=f"log{key}", name=f"log_{key}_{v}")
                    for key in ("Sp", "St", "Lp", "Lt")
                }
                all_logt.append(logt)
                # resolve combiners whose shifted reads land in this half
                for (dv, dlogt) in deferred:
                    pending.append(make_combiner(dv, dlogt, logt))
                deferred = []
                # last half of a row: shifted reads wrap to (r, 0), already
                # resident -> its own combine can start mid-scan, gated on
                # this half's logged columns
                own = make_combiner(v, logt, all_logt[r * cfg.nh]) \
                    if h == cfg.nh - 1 else None

                def ln_cb(si, st_cols, own=own):
                    # both longs logged after si >= 1 (stream order
                    # Lp, Lt, Sp, St); drip-feed so queued Abs ops never
                    # stall the next Ln in ACT's in-order queue
                    if si < 1:
                        return
                    budget = 8 if si < 3 else 12
                    if own is not None and st_cols and not own.done():
                        own(budget, st_cols)
                    for c in list(pending):
                        if budget <= 0:
                            break
                        c(budget)
                        if c.done():
                            pending.remove(c)
                        else:
                            budget = 0

                emit_passB(v, stateA[v][1], logt, ln_cb)
                stateA.pop(v - 1, None)
                if own is not None:
                    if not own.done():
                        pending.append(own)
                else:
                    deferred.append((v, logt))
            for c in pending:
                c(cfg.nslots)

            nc.sync.dma_start(part_d[:], partials[:])

    nc.compile()
    return nc


_CACHE = {}


def get_nc():
    if "nc" not in _CACHE:
        _CACHE["nc"] = build(Cfg())
    return _CACHE["nc"]


def make_in_maps(pred_sq, target_sq):
    rpc = B // NCORES
    return [
        {
            "xp": np.ascontiguousarray(pred_sq[k * rpc:(k + 1) * rpc]),
            "xt": np.ascontiguousarray(target_sq[k * rpc:(k + 1) * rpc]),
        }
        for k in range(NCORES)
    ]


def reduce_results(results):
    tot = 0.0
    for r in results:
        tot += float(r["partials"].astype(np.float64).sum())
    return np.float32(tot / (B * T))


def kernel(pred_sq, target_sq):
    from concourse.bass_utils import run_bass_kernel_spmd

    nc = get_nc()
    res = run_bass_kernel_spmd(nc, make_in_maps(pred_sq, target_sq),
                               core_ids=list(range(NCORES)))
    return reduce_results(res.results)

